# revision 11
# baseline (speedup 1.0000x reference)
"""LoRA grouped-experts MoE MLP on 8 NeuronCores (expert-parallel).

The LoRA adapters are folded into the base weights on the host
(W_eff = W + (alpha/r) * A @ B, computed in fp32, cast to bf16 — exact
to well under the bf16 quantization noise), so each core runs a plain
3-matmul MLP for one expert:
    g = silu(x @ Wg'), u = x @ Wu', h = g * u, o = h @ Wd'

Device layout (per core):
  - x is pre-transposed on host to xT [D, T] so the contraction dim D lands
    on SBUF partitions for both matmul operands (fp32 has no DMA transpose).
  - Layer 1 computes hT [H, T] (H on partitions). Layer 2 keeps the weight
    slices stationary and produces outT [D, T] in bf16; the host transposes
    back and widens to fp32.
  - All matmul inputs are bf16 (cast on host); PSUM accumulates fp32.
  - Every stationary (lhsT) operand feeds two back-to-back matmuls into two
    PSUM banks (the two 512-token halves), amortizing the serial LDWEIGHTS
    (measured ~165 ns per extra LDW when elision is disabled).
  - Weights stream through one shared slab pool ([128, 16, 1024] bf16 slabs
    = 2 KiB DRAM lines per partition); layer-2 prefetch overlaps layer-1.
  - bacc lowers each matmul to an explicit LDWEIGHTS + MATMULT; the
    _elide_redundant_ldw pass deletes the second, identical LDWEIGHTS of
    every pair post-compile when its semaphore waits are provably subsumed.

Perf notes (2026-08-11 hardware session):
  - The PE stream is the floor: 3072 N=512 bf16 matmuls = 1.57M PE cycles
    (~654 us at 2.4 GHz) + ~1536 partially-hidden LDWEIGHTS. Cold-chip
    8-core device time measures ~770 us/exec; sustained load thermally
    throttles the chip by up to ~25%, and 8-core operation adds ~80 us
    fixed + ~1.7 us/MB contention vs a single core.
  - fp8 (DoubleRow) was evaluated and rejected: pure e4m3 matmuls give
    7.5% rel err (gate 2e-2); error-compensated schemes lose more to
    un-hidden LDWEIGHTS than the 2x fp8 rate gains.
  - DMA layout is not the limiter: narrow/wide slabs, SP-only vs SP+Act
    DGE queues, and NO_WDMA probes all land within measurement noise.
"""

import os

import numpy as np
import ml_dtypes

import concourse.bacc as bacc
import concourse.mybir as mybir
import concourse.tile as tile
from concourse.bass import ts
from concourse.bass_utils import run_bass_kernel_spmd

P = 128
E, D, H, R, T = 8, 2048, 4096, 16, 1024
DO = D // P   # 16
HO = H // P   # 32
ALPHA = 32.0
BF16 = mybir.dt.bfloat16
F32 = mybir.dt.float32

_NC_CACHE = []
LAST_RESULT = None

NSPLIT = int(os.environ.get("KERNEL_NSPLIT", "4"))
OSPLIT = os.environ.get("KERNEL_OSPLIT", "0") == "1"  # o-halved slab tiles
WBUFS = int(os.environ.get("KERNEL_WBUFS", "6" if OSPLIT else "3"))
WIDE = os.environ.get("KERNEL_WIDE", "1") == "1"  # 1024-wide (2KB-line) slabs
DMA_SPLIT_ENGINES = os.environ.get("KERNEL_DMA_ENG", "1") == "1"
LDW_ELIDE = os.environ.get("KERNEL_LDW_ELIDE", "1") == "1"
NO_WDMA = os.environ.get("KERNEL_NO_WDMA", "0") == "1"  # timing probe only
OUT_F32 = os.environ.get("KERNEL_OUT_F32", "0") == "1"
STAGGER = os.environ.get("KERNEL_STAGGER", "0") == "1"


def _mm_pair(nc, tc, out0, out1, lhsT, rhs0, rhs1, start, stop):
    """Two matmuls sharing one stationary operand, emitted back to back so
    the post-compile _elide_redundant_ldw pass can drop the second reload."""
    nc.tensor.matmul(out0, lhsT, rhs0, start=start, stop=stop)
    nc.tensor.matmul(out1, lhsT, rhs1, start=start, stop=stop)


def _wap_key(inst, idx):
    import json
    d = json.loads(mybir.instruction_to_pretty_json_string(inst))
    return json.dumps(d["ins"][idx], sort_keys=True)


def _elide_redundant_ldw(nc):
    """bacc compiles every nc.tensor.matmul into an explicit InstLdweights +
    InstMatmult pair, so a stationary operand feeding two matmuls is loaded
    twice. Delete a reload when provably safe:

      * its weights AP is byte-identical to the previous (kept) LDW and
        exactly one matmul consumed that load, and
      * it carries no semaphore updates, and
      * every one of its sem-ge waits is already subsumed by a wait executed
        earlier in this block's PE stream (sem values are monotone within a
        block; loop resets live in other blocks).

    Returns the number of deleted LDWs."""
    deleted = 0
    for fn in nc.m.functions:
        for bb in fn.blocks:
            waited = {}  # sem id -> max value already waited on PE stream
            last_key = None
            mms_since = 0
            to_delete = set()

            def note_waits(inst):
                si = inst.sync_info
                if si is None:
                    return
                for w in si.on_wait:
                    if w.wait_mode == "sem-ge-imm" and w.wait_value is not None:
                        if waited.get(w.id, -1) < w.wait_value:
                            waited[w.id] = w.wait_value
            for inst in bb.instructions:
                if inst.engine != mybir.EngineType.PE:
                    continue
                if isinstance(inst, mybir.InstLdweights):
                    key = _wap_key(inst, 0)
                    si = inst.sync_info
                    on_wait = si.on_wait if si is not None else []
                    on_update = si.on_update if si is not None else []
                    subsumed = all(
                        w.wait_mode == "sem-ge-imm"
                        and w.wait_value is not None
                        and waited.get(w.id, -1) >= w.wait_value
                        for w in on_wait)
                    if (key == last_key and mms_since == 1
                            and not on_update and subsumed):
                        to_delete.add(id(inst))
                        deleted += 1
                        mms_since = 0
                        continue
                    note_waits(inst)
                    last_key = key
                    mms_since = 0
                else:
                    note_waits(inst)
                    if isinstance(inst, mybir.InstMatmult):
                        mms_since += 1
            if to_delete:
                bb.instructions = [i for i in bb.instructions
                                   if id(i) not in to_delete]
    return deleted


def _validate_ldw_elision(nc):
    """Post-elision check over the final stream: at every matmul, the PE
    array must hold exactly that matmul's weights AP."""
    n_mm = 0
    for fn in nc.m.functions:
        for bb in fn.blocks:
            last_w = None
            for inst in bb.instructions:
                if isinstance(inst, mybir.InstLdweights):
                    last_w = _wap_key(inst, 0)
                elif isinstance(inst, mybir.InstMatmult):
                    n_mm += 1
                    wap = _wap_key(inst, 1)
                    if last_w != wap:
                        raise RuntimeError(
                            f"ldw elision broke {inst.name}: array holds "
                            f"{last_w}, matmul needs {wap}")
    return n_mm


def _build_nc(reps=1, loop_n=None):
    nc = bacc.Bacc("TRN2", target_bir_lowering=False, debug=False, num_devices=E)

    out_dt = F32 if OUT_F32 else BF16
    xT = nc.dram_tensor("xT", (D, T), BF16, kind="ExternalInput").ap()
    wg = nc.dram_tensor("wg", (D, H), BF16, kind="ExternalInput").ap()
    wu = nc.dram_tensor("wu", (D, H), BF16, kind="ExternalInput").ap()
    wd = nc.dram_tensor("wd", (H, D), BF16, kind="ExternalInput").ap()
    out = nc.dram_tensor("out", (D, T), out_dt, kind="ExternalOutput").ap()

    aps = dict(
        xT_r=xT.rearrange("(o p) t -> p o t", p=P),
        wg_r=wg.rearrange("(o p) h -> p o h", p=P),
        wu_r=wu.rearrange("(o p) h -> p o h", p=P),
        wd_r=wd.rearrange("(o p) d -> p o d", p=P),
        out_r=out.rearrange("(o p) t -> p o t", p=P),
        out_dt=out_dt,
    )

    with tile.TileContext(nc) as tc:
        with (
            tc.tile_pool(name="persist", bufs=1) as pp,
            tc.tile_pool(name="stage", bufs=3) as sp,
            tc.tile_pool(name="wpool", bufs=WBUFS) as wp,
            tc.tile_pool(name="psum", bufs=8, space="PSUM") as psp,
        ):
            if loop_n is not None:
                kw = {}
                if STAGGER:
                    # avoid the full-engine drain barrier at each loop
                    # iteration (sem resets staggered per engine instead)
                    kw = dict(staggered_reset=True,
                              hint_engines=tuple(mybir.ALL_ENGINES))
                with tc.For_i(0, loop_n, 1, **kw):
                    for rep in range(reps):
                        _emit(nc, tc, pp, sp, wp, psp, aps, rep)
            else:
                for rep in range(reps):
                    _emit(nc, tc, pp, sp, wp, psp, aps, rep)

    nc.compile()
    if LDW_ELIDE:
        n_del = _elide_redundant_ldw(nc)
        _validate_ldw_elision(nc)
        assert n_del > 0, "LDW_ELIDE on but nothing elided"
    return nc


_DMA_RR = [0]


def _dma_eng(nc):
    """Round-robin input DMAs over the two HWDGE-capable engines (SP, Act)
    so transfers spread across two DGE queues."""
    if not DMA_SPLIT_ENGINES:
        return nc.sync
    _DMA_RR[0] ^= 1
    return nc.sync if _DMA_RR[0] else nc.scalar


def _dma_split(nc, dst, src, n):
    """Split a [P, O, F] slab load into n dma_starts over the O axis."""
    if NO_WDMA:
        # timing probe: load 1/16 of the slab so tiles stay allocated but
        # ~94% of the weight DMA traffic disappears
        nc.sync.dma_start(dst[:, 0:1, :], src[:, 0:1, :])
        return
    n = max(1, min(n, NSPLIT)) if NSPLIT > 0 else 1
    o = dst.shape[1]
    step = o // n
    for i in range(n):
        _dma_eng(nc).dma_start(dst[:, ts(i, step), :], src[:, ts(i, step), :])


class _SlabPair:
    """Two o-halved tiles presented with the full-slab indexing convention,
    so matmul chains can start as soon as the low half has landed."""

    def __init__(self, lo, hi, osplit):
        self.lo, self.hi, self.osplit = lo, hi, osplit

    def __getitem__(self, idx):
        p, o, cols = idx
        t = self.lo if o < self.osplit else self.hi
        return t[p, o % self.osplit, cols]


def _load_slab(nc, wp, src, sw):
    """Load a [P, DO, sw] weight slab; with OSPLIT, as two [P, DO/2, sw]
    tiles (finer prefetch granularity, same bytes and DMA count)."""
    if not OSPLIT:
        t = wp.tile([P, DO, sw], BF16, tag="w")
        _dma_split(nc, t, src, 4)
        return t
    half = DO // 2
    lo = wp.tile([P, half, sw], BF16, tag="w")
    _dma_split(nc, lo, src[:, 0:half, :], 2)
    hi = wp.tile([P, half, sw], BF16, tag="w")
    _dma_split(nc, hi, src[:, half:DO, :], 2)
    return _SlabPair(lo, hi, half)


def _emit(nc, tc, pp, sp, wp, psp, aps, rep):
    xT_r, wg_r, wu_r, wd_r = aps["xT_r"], aps["wg_r"], aps["wu_r"], aps["wd_r"]
    out_r, out_dt = aps["out_r"], aps["out_dt"]

    # slab width in h/d columns: 1024 gives 2 KiB DRAM lines per partition
    SW = 1024 if WIDE else 512
    NS = SW // P  # stationary 128-col slices per slab

    hT_sb = pp.tile([P, HO, T], BF16, tag="hT")

    with tc.tile_pool(name=f"xpool{rep}", bufs=1) as xp:
        xT_sb = xp.tile([P, DO, T], BF16, tag="xT")
        _dma_split(nc, xT_sb, xT_r, 4)

        # layer 1: hT[h, t] = silu(gate) * up; lhsT paired over t-halves
        for j in range(H // SW):
            wg_t = _load_slab(nc, wp, wg_r[:, :, ts(j, SW)], SW)
            wu_t = _load_slab(nc, wp, wu_r[:, :, ts(j, SW)], SW)
            for hsub in range(NS):
                hc = j * NS + hsub

                def l1_proj(w_t):
                    p0 = psp.tile([P, 512], F32, tag="mm")
                    p1 = psp.tile([P, 512], F32, tag="mm")
                    for o in range(DO):
                        _mm_pair(nc, tc, p0[:], p1[:], w_t[:, o, ts(hsub, P)],
                                 xT_sb[:, o, 0:512], xT_sb[:, o, 512:1024],
                                 o == 0, o == DO - 1)
                    return p0, p1

                pg0, pg1 = l1_proj(wg_t)
                pu0, pu1 = l1_proj(wu_t)
                for t, pg_, pu_ in ((0, pg0, pu0), (1, pg1, pu1)):
                    g_act = sp.tile([P, 512], F32, tag="gact")
                    nc.scalar.activation(
                        g_act[:], pg_[:], mybir.ActivationFunctionType.Silu)
                    nc.vector.tensor_mul(
                        hT_sb[:, hc, ts(t, 512)], g_act[:], pu_[:])

    # layer 2: outT[d, t] = (h @ Wd)^T; weight slices stationary,
    # paired over t-halves.
    for k in range(D // SW):
        s0 = _load_slab(nc, wp, wd_r[:, 0:16, ts(k, SW)], SW)
        s1 = _load_slab(nc, wp, wd_r[:, 16:32, ts(k, SW)], SW)
        for dsub in range(NS):
            dd = k * NS + dsub  # global 128-wide d-chunk
            po0 = psp.tile([P, 512], F32, tag="mm")
            po1 = psp.tile([P, 512], F32, tag="mm")
            for hc in range(HO):
                lhsT = (s0 if hc < 16 else s1)[:, hc % 16, ts(dsub, P)]
                _mm_pair(nc, tc, po0[:], po1[:], lhsT,
                         hT_sb[:, hc, 0:512], hT_sb[:, hc, 512:1024],
                         hc == 0, hc == HO - 1)
            o_t = sp.tile([P, T], out_dt, tag="ostage")
            nc.scalar.copy(o_t[:, 0:512], po0[:])
            nc.scalar.copy(o_t[:, 512:1024], po1[:])
            nc.sync.dma_start(out_r[:, dd, :], o_t[:])


def _get_nc():
    if not _NC_CACHE:
        _NC_CACHE.append(_build_nc())
    return _NC_CACHE[0]


def make_in_maps(x, gate_proj, up_proj, down_proj, lga, lgb, lua, lub, lda, ldb):
    """Host-side shard/fold/cast prep, shared by kernel() and the bench
    harness. Folds each LoRA pair into its base weight in fp32."""
    bf = ml_dtypes.bfloat16
    scale = ALPHA / R
    x = np.asarray(x, np.float32).reshape(E, T, D)

    def fold(w, a, b):
        w = np.asarray(w, np.float32)
        a = np.asarray(a, np.float32)
        b = np.asarray(b, np.float32)
        return (w + scale * (a @ b)).astype(bf)

    in_maps = []
    for e in range(E):
        in_maps.append({
            "xT": np.ascontiguousarray(x[e].T).astype(bf),
            "wg": fold(gate_proj[e], lga[e], lgb[e]),
            "wu": fold(up_proj[e], lua[e], lub[e]),
            "wd": fold(down_proj[e], lda[e], ldb[e]),
        })
    return in_maps


def kernel(x, num_tokens_per_expert, gate_proj, up_proj, down_proj,
           lora_gate_a, lora_gate_b, lora_up_a, lora_up_b,
           lora_down_a, lora_down_b):
    global LAST_RESULT
    in_maps = make_in_maps(x, gate_proj, up_proj, down_proj,
                           lora_gate_a, lora_gate_b, lora_up_a, lora_up_b,
                           lora_down_a, lora_down_b)
    # The axon NTFF profile hook is unavailable in this container; force the
    # no-trace PJRT path regardless of ambient BASS_TRACE.
    os.environ["BASS_NEVER_TRACE"] = "1"
    nc = _get_nc()
    res = run_bass_kernel_spmd(nc, in_maps, core_ids=list(range(E)))
    LAST_RESULT = res
    # outputs are outT [D, T] per expert; transpose back to [T, D]
    return np.concatenate(
        [np.ascontiguousarray(r["out"].T).astype(np.float32)
         for r in res.results], axis=0)
